# revision 42
# baseline (speedup 1.0000x reference)
"""Multi-head self-attention (B=8, N=1024, C=768, H=12) on 8 trn2 NeuronCores.

Sharding: data-parallel over batch — core b computes batch element b end to
end; weights are replicated. No collectives.

Per-core dataflow (all matmuls on TensorE, out = lhsT.T @ rhs, contraction on
the partition dim):

  1. Weights are host-prearranged into dense per-need layouts so every DMA
     descriptor is a contiguous 1.5-2KB line (strided 256B-segment transfers
     crawl at ~60GB/s): wqk [P, t-major (t,k,128)] for the 12 Q/K c'-tiles,
     wv [P, k-major (k,768)], wp [P, k-major (k,768)].  Transfers are issued
     in need order across the three DMA queues (scalar/sync/gpsimd).
  2. qkv^T for Q,K in [c', n] layout as fine-grained units: one 6-matmul
     accumulation run per (c'-tile, 512-col half) into a [128,512] PSUM
     bank, bias fused into the PSUM->SBUF copy on DVE.
  3. V in token-major per-head blocks [ones(64) | V_h] (128 cols per head):
     the 64 ones columns make the A@V matmul produce the softmax row-sums
     replicated across 64 partitions, so normalization needs no partition
     broadcast.  V bias is skipped on-device: since softmax rows sum to 1,
     it folds into an adjusted proj bias bp' = b_proj + W_proj @ b_qkv[V]
     (host-computed).
  4. Per head pair p, slot s -> (mt, g): S^T[m, n] = (K_h^T) @ Q_h^T for
     both heads concurrently in the two PE row-quadrants (K = d = 64). exp
     via ScalarE reading PSUM (2-deep [128,1024] rotation), writing SBUF
     bf16 (scale folded into the K projection host-side; max-subtraction
     skipped — scores are O(1) and softmax is shift-invariant).  The exp
     table is preloaded via a dummy 1-col exp during the DMA dead zone.
  5. AV: one 8-matmul consecutive same-bank accumulation run per (head, g)
     into a [128,512] bank from a 4-deep PSUM pool, normalized eagerly
     (custom-DVE fast reciprocal on the replicated sums at PSUM base
     partition 0, then tensor_mul into ouT[c, n]) so the bank frees fast.
  6. proj in two stages: projA = j=0..3 k-tiles -> ysb[nt] (+bias), run
     inside pairs 4-5; tail = j=4,5 + DVE merge with ysb + y DMA (bf16,
     spread over 3 DMA queues).
  7. Pair 5 runs its slots g-major (all g=0 then all g=1) so its AV(g=0),
     normalize, and the nt=0..3 tail units overlap the g=1 exps; only the
     g=1 AV/normalize/tails remain after the last exp.

Scheduling: after each score+exp emission, filler units (each one PSUM-bank
accumulation run + one DVE drain) are drained from a per-pair list by
cumulative time-budget pacing with per-unit earliest-slot constraints
matching DMA arrival and dependency readiness.

All matmul inputs are bf16 (fp32 accumulate); y is written bf16 and upcast
on host.
"""

import numpy as np
import ml_dtypes

B, N, C = 8, 1024, 768
H, D = 12, 64
HB = 2 * D  # per-head V block width: [ones(64) | V_h(64)]
N_CORES = 8
P = 128
KT = C // P  # 6 contraction tiles
NT = N // P  # 8 token tiles
NQT = 2 * C // P  # 12 q/k c'-tiles; pair p uses tiles p and 6+p

_CACHE: dict = {}

MM512 = 215  # ns, warm 512-col bf16 matmul issue-to-issue
MM256 = 110

# x k-tile DMA arrival order (see queue assignment below)
KS_ORDER = [0, 4, 1, 5, 2, 3]
# wv k-chunk arrival order
VORDER = [0, 1, 2, 3, 4, 5]
# wqk c'-tile order in the host-packed tensor: need order (pair 0 first)
TORDER = [0, 6, 1, 7, 2, 8, 3, 9, 4, 10, 5, 11]
TPOS = {t: i for i, t in enumerate(TORDER)}


def _build(cfg: dict):
    import concourse.bass as bass
    import concourse.bacc as bacc
    import concourse.mybir as mybir
    import concourse.tile as tile

    dt = mybir.dt
    f32 = dt.float32
    bf16 = dt.bfloat16

    nc = bacc.Bacc("TRN2", target_bir_lowering=False, debug=False,
                   num_devices=N_CORES)

    xT_d = nc.dram_tensor("xT", [P, KT * N], bf16, kind="ExternalInput")
    wqk_d = nc.dram_tensor("wqk", [P, NQT * KT * P], bf16,
                           kind="ExternalInput")
    wv_d = nc.dram_tensor("wv", [P, KT * C], bf16, kind="ExternalInput")
    wp_d = nc.dram_tensor("wp", [P, KT * C], bf16, kind="ExternalInput")
    bqk_d = nc.dram_tensor("bqk", [P, NQT], f32, kind="ExternalInput")
    bp_d = nc.dram_tensor("bp", [1, C], f32, kind="ExternalInput")
    ident_d = nc.dram_tensor("ident", [P, P], bf16, kind="ExternalInput")
    y_d = nc.dram_tensor("y", [N, C], bf16, kind="ExternalOutput")

    with tile.TileContext(nc, pool_alloc_mode="queue") as tc:
        with (
            tc.tile_pool(name="const", bufs=1) as cpool,
            tc.tile_pool(name="work", bufs=2) as workpool,
            tc.tile_pool(name="ps_s", bufs=2, space="PSUM") as ps_s,
            tc.tile_pool(name="ps_f", bufs=4, space="PSUM") as ps_f,
        ):
            # ---- persistent SBUF inputs ----
            wqk1 = cpool.tile([P, NQT * KT * P], bf16, name="wqk1",
                              tag="wqk1")
            wv1 = cpool.tile([P, KT * C], bf16, name="wv1", tag="wv1")
            wp1 = cpool.tile([P, KT * C], bf16, name="wp1", tag="wp1")
            bqk = cpool.tile([P, NQT], f32, name="bqk", tag="bqk")
            bp = cpool.tile([1, C], f32, name="bp", tag="bp")
            xT1 = cpool.tile([P, KT * N], bf16, name="xT1", tag="xT1")
            ident = cpool.tile([P, P], bf16, name="ident", tag="ident")
            zt = cpool.tile([P, 512], bf16, name="zt", tag="zt")
            nc.vector.memset(zt[:], 0.0)

            def xdma(eng, k0, k1):
                # x k-tiles k0..k1-1 in one dense transfer (4KB+ rows)
                eng.dma_start(xT1[:, k0 * N:k1 * N],
                              xT_d.ap()[:, k0 * N:k1 * N])

            def tdma(eng, i0, i1):
                # packed c'-tiles TORDER[i0..i1-1] in one dense transfer
                w = KT * P
                eng.dma_start(wqk1[:, i0 * w:i1 * w],
                              wqk_d.ap()[:, i0 * w:i1 * w])

            def vdma(eng, k0, k1):
                eng.dma_start(wv1[:, k0 * C:k1 * C],
                              wv_d.ap()[:, k0 * C:k1 * C])

            # sync queue (earliest start): t0/t6 first, then x k3, t1/t7
            # and the late-deadline tiles t4/t10/t5/t11
            tdma(nc.sync, 0, 2)
            xdma(nc.sync, 3, 4)
            tdma(nc.sync, 2, 4)
            tdma(nc.sync, 8, 12)
            # gpsimd queue: bias, x k4-5, t2/t8/t3/t9, V k3-5, proj weights
            nc.gpsimd.dma_start(bqk[:], bqk_d.ap())
            xdma(nc.gpsimd, 4, 6)
            tdma(nc.gpsimd, 4, 8)
            vdma(nc.gpsimd, 3, 6)
            nc.gpsimd.dma_start(bp[:], bp_d.ap())
            nc.gpsimd.dma_start(ident[:], ident_d.ap())
            nc.gpsimd.dma_start(wp1[:], wp_d.ap())
            # scalar queue (fast): x k0-2, exp-table preload in the
            # transfer dead zone, then V k0-2
            xdma(nc.scalar, 0, 3)
            dume = cpool.tile([P, 1], bf16, name="dume", tag="dume")
            nc.scalar.activation(dume[:], zt[:, 0:1],
                                 bass.mybir.ActivationFunctionType.Exp)
            vdma(nc.scalar, 0, 3)
            bp_b = cpool.tile([P, C], f32, name="bp_b", tag="bp_b")
            nc.gpsimd.partition_broadcast(bp_b[:], bp[:])
            # PE warm-up: junk matmuls during the DMA dead zone so the HAM
            # clock is ramped when real data lands (~13us in).
            jp = ps_s.tile([P, N], f32, name="jp", tag="s")
            for _ in range(20):
                nc.tensor.matmul(jp[:, 0:512], zt[:, 0:P], zt[:],
                                 start=True, stop=True)

            def xT(k):
                return xT1[:, k * N:(k + 1) * N]

            def wqk(t, k):
                i = TPOS[t] * KT + k
                return wqk1[:, i * P:(i + 1) * P]

            def wv(k):
                return wv1[:, k * C:(k + 1) * C]

            def wp(k):
                return wp1[:, k * C:(k + 1) * C]

            # ---- persistent SBUF intermediates ----
            qkT = [cpool.tile([P, N], bf16, name=f"qkT{t}", tag=f"qkT{t}")
                   for t in range(NQT)]
            etbig = [cpool.tile([P, NT * 2 * N], bf16, name=f"etbig{i}",
                                tag=f"etbig{i}") for i in range(2)]
            vbig = cpool.tile([P, NT * H * HB], bf16, name="vbig",
                              tag="vbig")
            v = [vbig[:, nt * H * HB:(nt + 1) * H * HB] for nt in range(NT)]
            ouT = [cpool.tile([P, N], bf16, name=f"ouT{j}", tag=f"ouT{j}")
                   for j in range(KT)]
            ysb = [cpool.tile([P, C], bf16, name=f"ysb{nt}", tag=f"ysb{nt}")
                   for nt in range(NT)]

            # ---- fine-grained filler units ----
            def qk_run(t, g, warm=False):
                # Q^T or K^T tile t, 512-col half g: 6-matmul accumulation
                # run; x k-tiles in DMA arrival order for the early tiles.
                # warm=True interleaves junk matmuls after each x-gated
                # member so the PE HAM clock stays ramped across the x-tile
                # DMA arrival gaps.
                ks = KS_ORDER if t in (0, 1, KT, KT + 1) else list(range(KT))
                pm = ps_f.tile([P, 512], f32, name="fm", tag="f")
                sl = slice(g * 512, (g + 1) * 512)
                for i, k in enumerate(ks):
                    nc.tensor.matmul(pm[:], wqk(t, k), xT(k)[:, sl],
                                     start=(i == 0), stop=(i == KT - 1))
                    if warm and i < KT - 1:
                        for _ in range(3):
                            nc.tensor.matmul(jp[:, 0:512], zt[:, 0:P],
                                             zt[:], start=True, stop=True)
                nc.vector.tensor_scalar_add(qkT[t][:, sl], pm[:],
                                            bqk[:, t:t + 1])

            def v_half(nt, half):
                # V for token tile nt, heads 0-7 (half 0) or 8-11 (half 1)
                dst = v[nt].rearrange("p (h c) -> p h c", c=HB)
                if half == 0:
                    nc.vector.memset(dst[:, :, 0:D], 1.0)
                off, width, h0, h1 = ((0, 512, 0, 8) if half == 0
                                      else (512, 256, 8, 12))
                pm = ps_f.tile([P, 512], f32, name="fm", tag="f")
                for i, k in enumerate(VORDER):
                    nc.tensor.matmul(
                        pm[:, 0:width],
                        xT(k)[:, nt * P:(nt + 1) * P],
                        wv(k)[:, off:off + width],
                        start=(i == 0), stop=(i == KT - 1),
                    )
                srcv = pm[:, 0:width].rearrange("p (h d) -> p h d", d=D)
                nc.vector.tensor_copy(dst[:, h0:h1, D:HB], srcv[:])

            def av_mms(st, par, g, pm, lo, hi, start, stop):
                # consecutive same-bank accumulation run over m-tiles
                # lo..hi-1 into pm
                h = 2 * st.p + par
                eb = etbig[st.p % 2]
                for i in range(lo, hi):
                    s_idx = st.slot_of(i, g)
                    nc.tensor.matmul(
                        pm[:],
                        v[i][:, h * HB:(h + 1) * HB],
                        eb[:, s_idx * N + par * 512:
                           s_idx * N + (par + 1) * 512],
                        start=(start and i == lo),
                        stop=(stop and i == hi - 1),
                    )

            def av_norm(st, par, g, pm):
                # eager normalize: sums replicated on partitions 0:64, O^T
                # on 64:128; recip reads PSUM at base partition 0.
                rb = workpool.tile([D, 512], f32, name="rb", tag="rb")
                nc.vector.reciprocal_approx_fast(rb[:], pm[0:D, :])
                nc.vector.tensor_mul(
                    ouT[st.p][par * D:(par + 1) * D,
                              g * 512:(g + 1) * 512],
                    pm[D:P, :], rb[:])

            def av_unit(st, par, g):
                pm = ps_f.tile([P, 512], f32, name="fm", tag="f")
                av_mms(st, par, g, pm, 0, NT, True, True)
                av_norm(st, par, g, pm)

            def projA(nt, off, width):
                # ysb[nt] <- sum_{j=0..3} ouT[j]^T @ wp[j] + bias
                pm = ps_f.tile([P, 512], f32, name="fm", tag="f")
                for j in range(4):
                    nc.tensor.matmul(
                        pm[:, 0:width],
                        ouT[j][:, nt * P:(nt + 1) * P],
                        wp(j)[:, off:off + width],
                        start=(j == 0), stop=(j == 3),
                    )
                nc.vector.tensor_add(ysb[nt][:, off:off + width],
                                     pm[:, 0:width],
                                     bp_b[:, off:off + width])

            dma_engines = [nc.sync, nc.scalar, nc.gpsimd]

            def tail_unit(nt):
                # j=4,5 contributions + DVE merge with ysb + y DMA (ACT is
                # exp-saturated at the tail, so drains go to DVE only).
                pm = ps_f.tile([P, 512], f32, name="fm", tag="f")
                pm2 = ps_f.tile([P, 512], f32, name="fm", tag="f")
                for ps, off, width in ((pm, 0, 512), (pm2, 512, 256)):
                    for j in (4, 5):
                        nc.tensor.matmul(
                            ps[:, 0:width],
                            ouT[j][:, nt * P:(nt + 1) * P],
                            wp(j)[:, off:off + width],
                            start=(j == 4), stop=(j == 5),
                        )
                yb = workpool.tile([P, C], bf16, name="yb", tag="yb",
                                   bufs=4)
                nc.vector.tensor_add(yb[:, 0:512], pm[:],
                                     ysb[nt][:, 0:512])
                nc.vector.tensor_add(yb[:, 512:C], pm2[:, 0:256],
                                     ysb[nt][:, 512:C])
                dma_engines[nt % 3].dma_start(
                    y_d.ap()[nt * P:(nt + 1) * P, :], yb[:])

            # ---- attention pieces ----
            class PairState:
                def __init__(self, p):
                    self.p = p
                    self.gmajor = (p == H // 2 - 1)

                def slot_of(self, mt, g):
                    return g * NT + mt if self.gmajor else 2 * mt + g

            def score_exp(st, mt, g):
                qt = qkT[st.p]
                kt = qkT[NQT // 2 + st.p]
                sp = ps_s.tile([P, N], f32, name="sp", tag="s")
                for par in range(2):
                    o = par * D
                    nc.tensor.matmul(
                        sp[:, par * 512:(par + 1) * 512],
                        kt[o:o + D, mt * P:(mt + 1) * P],
                        qt[o:o + D, g * 512:(g + 1) * 512],
                        start=True, stop=True,
                    )
                s_idx = st.slot_of(mt, g)
                et = etbig[st.p % 2][:, s_idx * N:(s_idx + 1) * N]
                nc.scalar.activation(
                    et, sp[:], bass.mybir.ActivationFunctionType.Exp)

            # ---- per-pair filler unit lists: (min_slot, cost_ns, fn) ----
            NPAIR = H // 2
            units: list = [[] for _ in range(NPAIR)]
            UQK = 6 * MM512 + 80
            UV0 = 6 * MM512 + 80
            UV1 = 6 * MM256 + 80
            UAV = 8 * MM512 + 80
            UPA = {512: 4 * MM512 + 80, 256: 4 * MM256 + 80}
            UTL = 2 * MM512 + 2 * MM256 + 80

            def add(p, ms, cost, fn):
                units[p].append((ms, cost, fn))

            # pair 0: pair-1 qk tiles early (t1/t7 land ~14-18us), V half0
            # as the wv chunks land, vh1 x2 late.
            add(0, 0, UQK, lambda: qk_run(0, 1))
            add(0, 1, UQK, lambda: qk_run(6, 1))
            add(0, 2, UQK, lambda: qk_run(1, 0))
            add(0, 3, UQK, lambda: qk_run(7, 0))
            add(0, 4, UQK, lambda: qk_run(1, 1))
            add(0, 5, UQK, lambda: qk_run(7, 1))
            for nt in range(NT):
                add(0, nt + 3, UV0, lambda nt=nt: v_half(nt, 0))
            # pairs 1-4: AV(prev) x4 from slot 2, hosted qk tiles, vh1,
            # projA in pair 4.
            for p in range(1, 5):
                for i, (par, g) in enumerate(
                        ((0, 0), (1, 0), (0, 1), (1, 1))):
                    add(p, 2 + i, UAV,
                        lambda par=par, g=g: ("av_prev", par, g))
            for i, (t, g) in enumerate(((2, 0), (8, 0), (2, 1), (8, 1))):
                add(1, 1 + i, UQK, lambda t=t, g=g: qk_run(t, g))
            for nt in (0, 1, 2):
                add(1, 0, UV1, lambda nt=nt: v_half(nt, 1))
            add(1, 5, UQK, lambda: qk_run(5, 0))
            for i, (t, g) in enumerate(((3, 0), (9, 0), (3, 1), (9, 1))):
                add(2, 1 + i, UQK, lambda t=t, g=g: qk_run(t, g))
            for nt in (3, 4, 5):
                add(2, 0, UV1, lambda nt=nt: v_half(nt, 1))
            add(2, 5, UQK, lambda: qk_run(5, 1))
            for i, (t, g) in enumerate(((4, 0), (10, 0), (4, 1), (10, 1))):
                add(3, 1 + i, UQK, lambda t=t, g=g: qk_run(t, g))
            for nt in (6, 7):
                add(3, 0, UV1, lambda nt=nt: v_half(nt, 1))
            add(3, 5, UQK, lambda: qk_run(11, 0))
            add(3, 6, UQK, lambda: qk_run(11, 1))
            for nt in range(6):
                add(4, 6, UPA[512], lambda nt=nt: projA(nt, 0, 512))
                add(4, 6, UPA[256], lambda nt=nt: projA(nt, 512, 256))
            # pair 5 (g-major slots): AV(p4) x4, projA nt 6-7, AV(p5,g0)
            # after slot 9, then the nt 0-3 tails.
            for i, (par, g) in enumerate(((0, 0), (1, 0), (0, 1), (1, 1))):
                add(5, 2 + i, UAV, lambda par=par, g=g: ("av_prev", par, g))
            for nt in (6, 7):
                add(5, 2, UPA[512], lambda nt=nt: projA(nt, 0, 512))
                add(5, 2, UPA[256], lambda nt=nt: projA(nt, 512, 256))
            add(5, 10, UAV, lambda: ("av_cur", 0, 0))
            add(5, 11, UAV, lambda: ("av_cur", 1, 0))
            for nt in range(4):
                add(5, 13, UTL, lambda nt=nt: tail_unit(nt))


            # pair-0 q/k g0 halves up front — the first score slot
            # unblocks right after these two units drain
            qk_run(0, 0)
            qk_run(6, 0)

            # ---- main loop ----
            prev = None
            for p in range(NPAIR):
                cur = PairState(p)
                ulist = units[p]
                total = sum(c for _, c, _ in ulist) + 16 * MM512
                spent = 0

                def emit(fn):
                    r = fn()
                    if isinstance(r, tuple):
                        av_unit(prev if r[0] == "av_prev" else cur,
                                r[1], r[2])
                for s in range(16):
                    if cur.gmajor:
                        g, mt = divmod(s, NT)
                    else:
                        mt, g = divmod(s, 2)
                    score_exp(cur, mt, g)
                    spent += MM512
                    budget = total * (s + 1) // 16
                    while ulist:
                        idx = next((i for i, u in enumerate(ulist)
                                    if u[0] <= s), None)
                        if idx is None or spent > budget:
                            break
                        _, c, fn = ulist.pop(idx)
                        emit(fn)
                        spent += c
                for _, c, fn in ulist:
                    emit(fn)
                prev = cur

            # ---- tail: pair-5 g=1 AV + normalize + nt 4-7 tails ----
            for par in range(2):
                av_unit(prev, par, 1)
            for nt in range(4, NT):
                tail_unit(nt)

    nc.compile()
    return nc


DEFAULT_CFG = dict()


def _host_prep(x, W_qkv, b_qkv, W_proj, b_proj, cfg):
    """Shard + lay out host-side numpy inputs per core."""
    scale = np.float32(1.0 / np.sqrt(D))
    wqkvT = np.ascontiguousarray(W_qkv.T).astype(np.float32)
    # fold the 1/sqrt(D) score scale into the K projection (cols C:2C)
    wqkvT[:, C:2 * C] *= scale
    wqkvT = wqkvT.astype(ml_dtypes.bfloat16)
    # dense per-need layouts (contiguous multi-KB line per partition row
    # per transfer chunk):
    # wqk[p, (t,k,128)]: c'-tile-major Q/K weights in TORDER need order
    wqk = np.empty((P, NQT * KT * P), dtype=ml_dtypes.bfloat16)
    for t in range(NQT):
        for k in range(KT):
            blk = wqkvT[k * P:(k + 1) * P, t * P:(t + 1) * P]
            i = TPOS[t] * KT + k
            wqk[:, i * P:(i + 1) * P] = blk
    # wv[p, (k,768)]: V weights k-major
    wv = np.empty((P, KT * C), dtype=ml_dtypes.bfloat16)
    for k in range(KT):
        wv[:, k * C:(k + 1) * C] = wqkvT[k * P:(k + 1) * P, 2 * C:3 * C]
    # wp[p, (k,768)]: proj weights k-major
    wprojT = np.ascontiguousarray(W_proj.T).astype(ml_dtypes.bfloat16)
    wp = np.empty((P, KT * C), dtype=ml_dtypes.bfloat16)
    for k in range(KT):
        wp[:, k * C:(k + 1) * C] = wprojT[k * P:(k + 1) * P, :]
    bqk_f = b_qkv[:2 * C].astype(np.float32).copy()
    bqk_f[C:2 * C] *= scale
    bqk = np.ascontiguousarray(bqk_f.reshape(NQT, P).T).astype(np.float32)
    bp_eff = (b_proj.astype(np.float64)
              + W_proj.astype(np.float64) @ b_qkv[2 * C:].astype(np.float64))
    bp = bp_eff.astype(np.float32).reshape(1, C)
    ident = np.eye(P, dtype=ml_dtypes.bfloat16)
    in_maps = []
    for b in range(N_CORES):
        # x^T prearranged k-major: xp[p, k*N + n] = x[b][n, k*P + p]
        xTb = np.ascontiguousarray(x[b].T).astype(ml_dtypes.bfloat16)
        xp = np.empty((P, KT * N), dtype=ml_dtypes.bfloat16)
        for k in range(KT):
            xp[:, k * N:(k + 1) * N] = xTb[k * P:(k + 1) * P, :]
        in_maps.append({"xT": xp, "wqk": wqk, "wv": wv, "wp": wp,
                        "bqk": bqk, "bp": bp, "ident": ident})
    return in_maps


def get_nc(cfg=None):
    cfg = dict(DEFAULT_CFG, **(cfg or {}))
    key = tuple(sorted(cfg.items()))
    if key not in _CACHE:
        _CACHE[key] = _build(cfg)
    return _CACHE[key]


def run(inputs, cfg=None, **run_kwargs):
    from concourse import bass_utils

    cfg = dict(DEFAULT_CFG, **(cfg or {}))
    nc = get_nc(cfg)
    in_maps = _host_prep(inputs["x"], inputs["W_qkv"], inputs["b_qkv"],
                         inputs["W_proj"], inputs["b_proj"], cfg)
    res = bass_utils.run_bass_kernel_spmd(
        nc, in_maps, core_ids=list(range(N_CORES)), **run_kwargs)
    out = np.stack([res.results[b]["y"].astype(np.float32)
                    for b in range(N_CORES)], axis=0)
    return out, res


def kernel(**inputs) -> np.ndarray:
    inputs = {k: np.asarray(v) for k, v in inputs.items()}
    out, _ = run(inputs)
    return out
